# revision 63
# baseline (speedup 1.0000x reference)
"""MoE (8 routed experts top-2 + shared expert) Trainium2 kernel.

Expert-parallel sparse dispatch.  Top-2 routing is computed on host
(fp32 logits + fp64 softmax; the selection bit-matches the reference's
softmax->top_k because softmax is order-preserving and the minimum
2nd/3rd probability gap for these inputs is ~7e-5, far above fp32
matmul noise).  Core c computes:

  - routed expert c: the n_c tokens routed to it, gathered on host and
    padded to the uniform capacity CR = roundup(max_e n_e, 8); SwiGLU at
    d_expert=512, down-projected to a partial [CR, 1024] which is scaled
    per-token by the combine weight on the way out of PSUM;
  - shared-expert half (c%2): token quarter (c//2) through shared
    columns [512*(c%2) : 512*(c%2)+512], partial [512, 1024].

The host scatter-adds all partials into the full output (the unshard
step).  No collectives.  All matmuls are bf16 x bf16 -> fp32 PSUM
(rel err ~3e-3 against the 2e-2 gate).  All DRAM inputs are host
pre-permuted so each DMA reads one contiguous block per partition.
"""

import sys

sys.path.insert(0, "/opt/trn_rl_repo")

import ml_dtypes
import numpy as np

import concourse.tile as tile
import concourse.mybir as mybir
from concourse import bacc
from concourse.bass_utils import run_bass_kernel_spmd

F32 = mybir.dt.float32
BF16 = mybir.dt.bfloat16
ACT = mybir.ActivationFunctionType
ALU = mybir.AluOpType
NPBF = ml_dtypes.bfloat16

N_CORES = 8
D = 1024          # d_hidden
DE = 512          # d_expert (routed); also the shared-expert half width
E = 8             # routed experts
NS = 512          # shared-expert tokens per core (2048 / 4 quarters)
DC = D // 128     # 8 contraction chunks of 128
HC = DE // 128    # 4 expert-width chunks of 128


def _chunks(n):
    """Token chunks, ≤512 each, smallest (tail) first so the small-DMA
    overhead overlaps the fat chunks' compute."""
    ch = [(a, min(a + 512, n)) for a in range(0, n, 512)]
    return ch[::-1]


def build_program(CR):
    nc = bacc.Bacc(num_devices=N_CORES)

    # ---- per-core DRAM I/O (pre-permuted: partition dim first) ----
    xg_d = nc.dram_tensor("xg", [128, DC, CR], BF16, kind="ExternalInput")
    xs_d = nc.dram_tensor("xs", [128, DC, NS], BF16, kind="ExternalInput")
    cw_d = nc.dram_tensor("cw", [1, CR], F32, kind="ExternalInput")
    # [stack, hc, part, dc, col]; stack 0 = routed expert, 1 = shared half
    wg_d = nc.dram_tensor("wg", [2, HC, 128, DC, 128], BF16, kind="ExternalInput")
    wu_d = nc.dram_tensor("wu", [2, HC, 128, DC, 128], BF16, kind="ExternalInput")
    # [stack, part(h), hc, dcol]
    wd_d = nc.dram_tensor("wd", [2, 128, HC, D], BF16, kind="ExternalInput")
    outr_d = nc.dram_tensor("outr", [128, DC, CR], BF16, kind="ExternalOutput")
    outs_d = nc.dram_tensor("outs", [DC, 128, NS], BF16, kind="ExternalOutput")

    with tile.TileContext(nc) as tc:
        with (
            tc.tile_pool(name="xp", bufs=1) as xp,
            tc.tile_pool(name="wp", bufs=1) as wp,
            tc.tile_pool(name="wdp", bufs=1) as wdp,
            tc.tile_pool(name="hp", bufs=1) as hp,
            tc.tile_pool(name="sp", bufs=2) as sp,
            tc.tile_pool(name="op", bufs=4) as op,
            tc.tile_pool(name="psug", bufs=2, space="PSUM") as psug,
            tc.tile_pool(name="pso", bufs=4, space="PSUM") as pso,
        ):
            # ---- PE warmup: junk matmuls on a memset tile keep the HAM
            # activity window busy so the PE clock-gate is at 2.4 GHz (not
            # the idle-default 1.2) when the first real matmul issues.
            # 48 x 128-col matmuls end around t=12us, just as the first
            # real operands land; they depend on no DMA.
            wz = xp.tile([128, 128], BF16, name="wz")
            nc.vector.memset(wz[:], 0.0)
            for i in range(48):
                ps_w = pso.tile([128, 128], F32, tag="ps_o", name=f"warm{i}")
                nc.tensor.matmul(ps_w[:], wz[:], wz[:], start=True, stop=True)

            # ---- input loads, in the order compute needs them ----
            # first matmul group needs wg_r[0] + the (small) tail chunk of xg;
            # the first full upgate chunk is split at 128 so compute can
            # begin as soon as the first small xg pieces land
            ch_r = _chunks(CR)
            ch_up = []
            split_done = False
            for a, b in ch_r:
                if not split_done and b - a > 256:
                    ch_up += [(x, min(x + 128, b)) for x in range(a, b, 128)]
                    split_done = True
                else:
                    ch_up.append((a, b))
            xg_sb = xp.tile([128, DC, CR], BF16)
            wg_r = [
                wp.tile([128, DC, 128], BF16, tag=f"wg_r{h}", name=f"wg_r{h}")
                for h in range(HC)
            ]
            wu_r = [
                wp.tile([128, DC, 128], BF16, tag=f"wu_r{h}", name=f"wu_r{h}")
                for h in range(HC)
            ]
            # critical-path loads first, in the order compute consumes them
            nc.sync.dma_start(wg_r[0][:], wg_d[0, 0])
            a0, b0 = ch_up[0]
            nc.sync.dma_start(xg_sb[:, :, a0:b0], xg_d[:, :, a0:b0])
            nc.sync.dma_start(wu_r[0][:], wu_d[0, 0])
            for a, b in ch_up[1:]:
                nc.sync.dma_start(xg_sb[:, :, a:b], xg_d[:, :, a:b])
            for hc in range(1, HC):
                nc.sync.dma_start(wg_r[hc][:], wg_d[0, hc])
                nc.sync.dma_start(wu_r[hc][:], wu_d[0, hc])
            cw_sb = xp.tile([1, CR], F32)
            nc.sync.dma_start(cw_sb[:], cw_d[:])
            xs_sb = xp.tile([128, DC, NS], BF16)
            nc.sync.dma_start(xs_sb[:], xs_d[:])
            wg_s = []
            wu_s = []
            for hc in range(HC):
                g = wp.tile([128, DC, 128], BF16, tag=f"wg_s{hc}")
                nc.sync.dma_start(g[:], wg_d[1, hc])
                u = wp.tile([128, DC, 128], BF16, tag=f"wu_s{hc}")
                nc.sync.dma_start(u[:], wu_d[1, hc])
                wg_s.append(g)
                wu_s.append(u)
            wd_r = wdp.tile([128, HC, D], BF16, tag="wd_r")
            nc.sync.dma_start(wd_r[:], wd_d[0])
            wd_s = wdp.tile([128, HC, D], BF16, tag="wd_s")
            nc.sync.dma_start(wd_s[:], wd_d[1])

            cwb = xp.tile([128, CR], F32)
            nc.gpsimd.partition_broadcast(cwb[:], cw_sb[0:1, :])

            # ---- up/gate: h = silu(Wg.T x) * (Wu.T x) ----
            # chunks may differ per hc: hc0 runs while xg is still landing
            # (fine pieces), later hcs use fat chunks (less instr overhead)
            def mm_group(ps, w, x_sb, a, b):
                for c in range(DC):
                    nc.tensor.matmul(
                        ps[:],
                        w[:, c, :],
                        x_sb[:, c, a:b],
                        start=(c == 0),
                        stop=(c == DC - 1),
                    )

            def drain_ug(ps_g, ps_u, h_sb, hc, a, b):
                sil = sp.tile([128, b - a], F32, tag="sil", name="sil")
                nc.scalar.activation(sil[:], ps_g[:], ACT.Silu)
                nc.vector.tensor_tensor(
                    h_sb[:, hc, a:b], sil[:], ps_u[:], op=ALU.mult
                )

            def upgate(wg_l, wu_l, x_sb, h_sb, chunks_by_hc, lag_hc0=False):
                for hc in range(HC):
                    chunks = chunks_by_hc[hc]
                    if hc == 0 and lag_hc0 and len(chunks) > 1:
                        # gate groups run one chunk ahead of up groups, in
                        # the order the xg pieces + wu_r[0] arrive
                        gs = {}
                        for i, (a, b) in enumerate(chunks):
                            ps_g = psug.tile(
                                [128, b - a], F32, tag="ps_g", name=f"psg{i}"
                            )
                            mm_group(ps_g, wg_l[hc], x_sb, a, b)
                            gs[i] = (ps_g, a, b)
                            if i >= 1:
                                pg, aj, bj = gs[i - 1]
                                ps_u = psug.tile(
                                    [128, bj - aj], F32, tag="ps_u", name=f"psu{i-1}"
                                )
                                mm_group(ps_u, wu_l[hc], x_sb, aj, bj)
                                drain_ug(pg, ps_u, h_sb, hc, aj, bj)
                        pg, aj, bj = gs[len(chunks) - 1]
                        ps_u = psug.tile(
                            [128, bj - aj], F32, tag="ps_u", name="psu_last"
                        )
                        mm_group(ps_u, wu_l[hc], x_sb, aj, bj)
                        drain_ug(pg, ps_u, h_sb, hc, aj, bj)
                        continue
                    for a, b in chunks:
                        ps_g = psug.tile([128, b - a], F32, tag="ps_g")
                        ps_u = psug.tile([128, b - a], F32, tag="ps_u")
                        mm_group(ps_g, wg_l[hc], x_sb, a, b)
                        mm_group(ps_u, wu_l[hc], x_sb, a, b)
                        drain_ug(ps_g, ps_u, h_sb, hc, a, b)
            # ---- down projection; combine weight applied here (linear).
            # Routed outputs stage into one SBUF tile -> a single 1.1MB DMA
            # (overlapped by the shared down phase); shared outputs DMA
            # per-dc so the kernel tail stays short.
            def down_psum(wd_sb, h_sb, a, b, dc):
                ps_o = pso.tile([128, b - a], F32, tag="ps_o", name="ps_o")
                for hc in range(HC):
                    nc.tensor.matmul(
                        ps_o[:],
                        wd_sb[:, hc, dc * 128 : (dc + 1) * 128],
                        h_sb[:, hc, a:b],
                        start=(hc == 0),
                        stop=(hc == HC - 1),
                    )
                return ps_o

            h_r = hp.tile([128, HC, CR], BF16, tag="h_r")
            upgate(wg_r, wu_r, xg_sb, h_r, [ch_up] + [ch_r] * (HC - 1))
            h_s = hp.tile([128, HC, NS], BF16, tag="h_s")
            upgate(wg_s, wu_s, xs_sb, h_s, [_chunks(NS)] * HC)

            # PSUM drain: gpsimd cannot read PSUM, so vector does the
            # weighted routed drains; shared drains alternate scalar/vector
            # (scalar is idle after the silus)
            ost_r = hp.tile([128, DC, CR], BF16, tag="ost_r")
            for a, b in ch_r:
                for dc in range(DC):
                    ps_o = down_psum(wd_r, h_r, a, b, dc)
                    nc.vector.tensor_tensor(
                        ost_r[:, dc, a:b], ps_o[:], cwb[:, a:b], op=ALU.mult
                    )
            nc.sync.dma_start(outr_d[:], ost_r[:])
            for dc in range(DC):
                ps_o = down_psum(wd_s, h_s, 0, NS, dc)
                if dc == DC - 1:
                    # last drain split into two independent half tiles so the
                    # engines truly run in parallel (two writers to one tile
                    # serialize) and the halves DMA out as they finish
                    ost_a = op.tile([128, NS // 2], BF16, tag="ost", name="ost_a")
                    ost_b = op.tile([128, NS // 2], BF16, tag="ost", name="ost_b")
                    nc.scalar.activation(ost_a[:], ps_o[:, 0 : NS // 2], ACT.Copy)
                    nc.vector.tensor_copy(ost_b[:], ps_o[:, NS // 2 : NS])
                    nc.sync.dma_start(outs_d[dc][:, 0 : NS // 2], ost_a[:])
                    nc.sync.dma_start(outs_d[dc][:, NS // 2 : NS], ost_b[:])
                    continue
                ost = op.tile([128, NS], BF16, tag="ost")
                if dc % 2 == 0:
                    nc.scalar.activation(ost[:], ps_o[:], ACT.Copy)
                else:
                    nc.vector.tensor_copy(ost[:], ps_o[:])
                nc.sync.dma_start(outs_d[dc], ost[:])

    nc.compile()
    return nc


_NC_CACHE = {}


def _get_program(CR):
    if CR not in _NC_CACHE:
        _NC_CACHE[CR] = build_program(CR)
    return _NC_CACHE[CR]


def _route(xf, W_g):
    """Host top-2 routing: token lists + combine weights per expert."""
    logits = xf @ W_g                                   # [N, E] fp32
    l = logits.astype(np.float64)
    l -= l.max(axis=-1, keepdims=True)
    p = np.exp(l)
    p /= p.sum(axis=-1, keepdims=True)                  # fp64 softmax
    top2 = np.argsort(-logits, axis=-1, kind="stable")[:, :2]
    vals = np.take_along_axis(p, top2, axis=-1).astype(np.float32)
    idx, cw = [], []
    for e in range(E):
        mask = top2 == e                                # [N, 2]
        tok = np.nonzero(mask.any(axis=1))[0]
        w = vals[tok][mask[tok]]
        idx.append(tok)
        cw.append(w.astype(np.float32))
    return idx, cw


def _perm_x(m):
    """[1024, T] fp32 -> [128, DC, T] bf16 with row (c*128+p) at [p, c]."""
    return np.ascontiguousarray(
        m.astype(NPBF).reshape(DC, 128, -1).transpose(1, 0, 2)
    )


def _perm_w(m):
    """[1024, 512] -> [HC, 128, DC, 128]: [hc, p, dc, col] = m[dc*128+p, hc*128+col]."""
    return np.ascontiguousarray(
        m.astype(NPBF).reshape(DC, 128, HC, 128).transpose(2, 1, 0, 3)
    )


def _perm_wd(m):
    """[512, 1024] -> [128, HC, 1024]: [p, hc, d] = m[hc*128+p, d]."""
    return np.ascontiguousarray(
        m.astype(NPBF).reshape(HC, 128, D).transpose(1, 0, 2)
    )


def kernel(x, W_g, Wg_e, Wu_e, Wd_e, Wg_s, Wu_s, Wd_s, _trace=False, _trace_kwargs=None):
    x = np.asarray(x, dtype=np.float32)
    W_g = np.asarray(W_g, dtype=np.float32)
    Wg_e = np.asarray(Wg_e, dtype=np.float32)
    Wu_e = np.asarray(Wu_e, dtype=np.float32)
    Wd_e = np.asarray(Wd_e, dtype=np.float32)
    Wg_s = np.asarray(Wg_s, dtype=np.float32)
    Wu_s = np.asarray(Wu_s, dtype=np.float32)
    Wd_s = np.asarray(Wd_s, dtype=np.float32)

    B, T, _ = x.shape
    N = B * T
    xf = x.reshape(N, D)
    xT = np.ascontiguousarray(xf.T)                     # [D, N]

    idx, cw = _route(xf, W_g)
    CR = max(8, -(-max(len(i) for i in idx) // 8) * 8)
    nc = _get_program(CR)

    in_maps = []
    for c in range(N_CORES):
        e, half, q = c, c % 2, c // 2
        n_e = len(idx[e])
        xg = np.zeros((D, CR), dtype=np.float32)
        xg[:, :n_e] = xT[:, idx[e]]
        cwp = np.zeros((1, CR), dtype=np.float32)
        cwp[0, :n_e] = cw[e]
        in_maps.append(
            {
                "xg": _perm_x(xg),
                "xs": _perm_x(xT[:, q * NS : (q + 1) * NS]),
                "cw": cwp,
                "wg": np.stack(
                    [_perm_w(Wg_e[e]), _perm_w(Wg_s[:, half * DE : (half + 1) * DE])]
                ),
                "wu": np.stack(
                    [_perm_w(Wu_e[e]), _perm_w(Wu_s[:, half * DE : (half + 1) * DE])]
                ),
                "wd": np.stack(
                    [_perm_wd(Wd_e[e]), _perm_wd(Wd_s[half * DE : (half + 1) * DE, :])]
                ),
            }
        )

    res = run_bass_kernel_spmd(
        nc, in_maps, list(range(N_CORES)), trace=_trace, **(_trace_kwargs or {})
    )

    # ---- unshard: scatter-add partials into the full output ----
    out = np.zeros((N, D), dtype=np.float32)
    for c in range(N_CORES):
        e, q = c, c // 2
        n_e = len(idx[e])
        o_s = np.asarray(res.results[c]["outs"], dtype=np.float32).reshape(D, NS)
        out[q * NS : (q + 1) * NS, :] += o_s.T
        o_r = (
            np.asarray(res.results[c]["outr"], dtype=np.float32)
            .transpose(1, 0, 2)
            .reshape(D, CR)
        )
        out[idx[e], :] += o_r[:, :n_e].T
    result = out.reshape(B, T, D)
    if _trace:
        return result, res
    return result


# revision 64
# speedup vs baseline: 1.1651x; 1.1651x over previous
"""MoE (8 routed experts top-2 + shared expert) Trainium2 kernel.

Expert-parallel sparse dispatch.  Top-2 routing is computed on host
(fp32 logits + fp64 softmax; the selection bit-matches the reference's
softmax->top_k because softmax is order-preserving and the minimum
2nd/3rd probability gap for these inputs is ~7e-5, far above fp32
matmul noise).  Core c computes:

  - routed expert c: the n_c tokens routed to it, gathered on host and
    padded to the uniform capacity CR = roundup(max_e n_e, 8); SwiGLU at
    d_expert=512, down-projected to a partial [CR, 1024] which is scaled
    per-token by the combine weight on the way out of PSUM;
  - shared-expert half (c%2): token quarter (c//2) through shared
    columns [512*(c%2) : 512*(c%2)+512], partial [512, 1024].

The host scatter-adds all partials into the full output (the unshard
step).  No collectives.  All matmuls are bf16 x bf16 -> fp32 PSUM
(rel err ~3e-3 against the 2e-2 gate).  All DRAM inputs are host
pre-permuted so each DMA reads one contiguous block per partition.
"""

import sys

sys.path.insert(0, "/opt/trn_rl_repo")

import ml_dtypes
import numpy as np

import concourse.tile as tile
import concourse.mybir as mybir
from concourse import bacc
from concourse.bass_utils import run_bass_kernel_spmd

F32 = mybir.dt.float32
BF16 = mybir.dt.bfloat16
ACT = mybir.ActivationFunctionType
ALU = mybir.AluOpType
NPBF = ml_dtypes.bfloat16

N_CORES = 8
D = 1024          # d_hidden
DE = 512          # d_expert (routed); also the shared-expert half width
E = 8             # routed experts
NS = 512          # shared-expert tokens per core (2048 / 4 quarters)
DC = D // 128     # 8 contraction chunks of 128
HC = DE // 128    # 4 expert-width chunks of 128


def _chunks(n):
    """Token chunks, ≤512 each, smallest (tail) first so the small-DMA
    overhead overlaps the fat chunks' compute."""
    ch = [(a, min(a + 512, n)) for a in range(0, n, 512)]
    return ch[::-1]


def build_program(CR):
    nc = bacc.Bacc(num_devices=N_CORES)

    # ---- per-core DRAM I/O (pre-permuted: partition dim first) ----
    xg_d = nc.dram_tensor("xg", [128, DC, CR], BF16, kind="ExternalInput")
    xs_d = nc.dram_tensor("xs", [128, DC, NS], BF16, kind="ExternalInput")
    cw_d = nc.dram_tensor("cw", [1, CR], F32, kind="ExternalInput")
    # [stack, hc, part, dc, col]; stack 0 = routed expert, 1 = shared half
    wg_d = nc.dram_tensor("wg", [2, HC, 128, DC, 128], BF16, kind="ExternalInput")
    wu_d = nc.dram_tensor("wu", [2, HC, 128, DC, 128], BF16, kind="ExternalInput")
    # [stack, part(h), hc, dcol]
    wd_d = nc.dram_tensor("wd", [2, 128, HC, D], BF16, kind="ExternalInput")
    outr_d = nc.dram_tensor("outr", [128, DC, CR], BF16, kind="ExternalOutput")
    outs_d = nc.dram_tensor("outs", [DC, 128, NS], BF16, kind="ExternalOutput")

    with tile.TileContext(nc) as tc:
        with (
            tc.tile_pool(name="xp", bufs=1) as xp,
            tc.tile_pool(name="wp", bufs=1) as wp,
            tc.tile_pool(name="wdp", bufs=1) as wdp,
            tc.tile_pool(name="hp", bufs=1) as hp,
            tc.tile_pool(name="sp", bufs=2) as sp,
            tc.tile_pool(name="op", bufs=4) as op,
            tc.tile_pool(name="psug", bufs=2, space="PSUM") as psug,
            tc.tile_pool(name="pso", bufs=4, space="PSUM") as pso,
        ):
            # ---- PE warmup: junk matmuls on a memset tile keep the HAM
            # activity window busy so the PE clock-gate is at 2.4 GHz (not
            # the idle-default 1.2) when the first real matmul issues.
            # 48 x 128-col matmuls end around t=12us, just as the first
            # real operands land; they depend on no DMA.
            wz = xp.tile([128, 128], BF16, name="wz")
            nc.vector.memset(wz[:], 0.0)
            for i in range(48):
                ps_w = pso.tile([128, 128], F32, tag="ps_o", name=f"warm{i}")
                nc.tensor.matmul(ps_w[:], wz[:], wz[:], start=True, stop=True)

            # ---- input loads, in the order compute needs them ----
            # first matmul group needs wg_r[0] + the (small) tail chunk of xg;
            # the first full upgate chunk is split at 128 so compute can
            # begin as soon as the first small xg pieces land
            ch_r = _chunks(CR)
            ch_up = []
            split_done = False
            for a, b in ch_r:
                if not split_done and b - a > 256:
                    ch_up += [(x, min(x + 128, b)) for x in range(a, b, 128)]
                    split_done = True
                else:
                    ch_up.append((a, b))
            xg_sb = xp.tile([128, DC, CR], BF16)
            wg_r = [
                wp.tile([128, DC, 128], BF16, tag=f"wg_r{h}", name=f"wg_r{h}")
                for h in range(HC)
            ]
            wu_r = [
                wp.tile([128, DC, 128], BF16, tag=f"wu_r{h}", name=f"wu_r{h}")
                for h in range(HC)
            ]
            # critical-path loads first, in the order compute consumes them
            nc.sync.dma_start(wg_r[0][:], wg_d[0, 0])
            a0, b0 = ch_up[0]
            nc.sync.dma_start(xg_sb[:, :, a0:b0], xg_d[:, :, a0:b0])
            nc.sync.dma_start(wu_r[0][:], wu_d[0, 0])
            for a, b in ch_up[1:]:
                nc.sync.dma_start(xg_sb[:, :, a:b], xg_d[:, :, a:b])
            for hc in range(1, HC):
                nc.sync.dma_start(wg_r[hc][:], wg_d[0, hc])
                nc.sync.dma_start(wu_r[hc][:], wu_d[0, hc])
            cw_sb = xp.tile([1, CR], F32)
            nc.sync.dma_start(cw_sb[:], cw_d[:])
            xs_sb = xp.tile([128, DC, NS], BF16)
            nc.sync.dma_start(xs_sb[:], xs_d[:])
            wg_s = []
            wu_s = []
            for hc in range(HC):
                g = wp.tile([128, DC, 128], BF16, tag=f"wg_s{hc}")
                nc.sync.dma_start(g[:], wg_d[1, hc])
                u = wp.tile([128, DC, 128], BF16, tag=f"wu_s{hc}")
                nc.sync.dma_start(u[:], wu_d[1, hc])
                wg_s.append(g)
                wu_s.append(u)
            wd_r = wdp.tile([128, HC, D], BF16, tag="wd_r")
            nc.sync.dma_start(wd_r[:], wd_d[0])
            wd_s = wdp.tile([128, HC, D], BF16, tag="wd_s")
            nc.sync.dma_start(wd_s[:], wd_d[1])

            cwb = xp.tile([128, CR], F32)
            nc.gpsimd.partition_broadcast(cwb[:], cw_sb[0:1, :])

            # ---- up/gate: h = silu(Wg.T x) * (Wu.T x) ----
            # chunks may differ per hc: hc0 runs while xg is still landing
            # (fine pieces), later hcs use fat chunks (less instr overhead)
            def mm_group(ps, w, x_sb, a, b):
                for c in range(DC):
                    nc.tensor.matmul(
                        ps[:],
                        w[:, c, :],
                        x_sb[:, c, a:b],
                        start=(c == 0),
                        stop=(c == DC - 1),
                    )

            def drain_ug(ps_g, ps_u, h_sb, hc, a, b):
                sil = sp.tile([128, b - a], F32, tag="sil", name="sil")
                nc.scalar.activation(sil[:], ps_g[:], ACT.Silu)
                nc.vector.tensor_tensor(
                    h_sb[:, hc, a:b], sil[:], ps_u[:], op=ALU.mult
                )

            def upgate(wg_l, wu_l, x_sb, h_sb, chunks_by_hc, lag_hc0=False):
                for hc in range(HC):
                    chunks = chunks_by_hc[hc]
                    if hc == 0 and lag_hc0 and len(chunks) > 1:
                        # gate groups run one chunk ahead of up groups, in
                        # the order the xg pieces + wu_r[0] arrive
                        gs = {}
                        for i, (a, b) in enumerate(chunks):
                            ps_g = psug.tile(
                                [128, b - a], F32, tag="ps_g", name=f"psg{i}"
                            )
                            mm_group(ps_g, wg_l[hc], x_sb, a, b)
                            gs[i] = (ps_g, a, b)
                            if i >= 1:
                                pg, aj, bj = gs[i - 1]
                                ps_u = psug.tile(
                                    [128, bj - aj], F32, tag="ps_u", name=f"psu{i-1}"
                                )
                                mm_group(ps_u, wu_l[hc], x_sb, aj, bj)
                                drain_ug(pg, ps_u, h_sb, hc, aj, bj)
                        pg, aj, bj = gs[len(chunks) - 1]
                        ps_u = psug.tile(
                            [128, bj - aj], F32, tag="ps_u", name="psu_last"
                        )
                        mm_group(ps_u, wu_l[hc], x_sb, aj, bj)
                        drain_ug(pg, ps_u, h_sb, hc, aj, bj)
                        continue
                    for a, b in chunks:
                        ps_g = psug.tile([128, b - a], F32, tag="ps_g")
                        ps_u = psug.tile([128, b - a], F32, tag="ps_u")
                        mm_group(ps_g, wg_l[hc], x_sb, a, b)
                        mm_group(ps_u, wu_l[hc], x_sb, a, b)
                        drain_ug(ps_g, ps_u, h_sb, hc, a, b)
            # ---- down projection; combine weight applied here (linear).
            # Routed outputs stage into one SBUF tile -> a single 1.1MB DMA
            # (overlapped by the shared down phase); shared outputs DMA
            # per-dc so the kernel tail stays short.
            def down_psum(wd_sb, h_sb, a, b, dc):
                ps_o = pso.tile([128, b - a], F32, tag="ps_o", name="ps_o")
                for hc in range(HC):
                    nc.tensor.matmul(
                        ps_o[:],
                        wd_sb[:, hc, dc * 128 : (dc + 1) * 128],
                        h_sb[:, hc, a:b],
                        start=(hc == 0),
                        stop=(hc == HC - 1),
                    )
                return ps_o

            h_r = hp.tile([128, HC, CR], BF16, tag="h_r")
            upgate(wg_r, wu_r, xg_sb, h_r, [ch_up] + [ch_r] * (HC - 1))
            h_s = hp.tile([128, HC, NS], BF16, tag="h_s")
            upgate(wg_s, wu_s, xs_sb, h_s, [_chunks(NS)] * HC)

            # PSUM drain: gpsimd cannot read PSUM, so vector does the
            # weighted routed drains; shared drains alternate scalar/vector
            # (scalar is idle after the silus)
            ost_r = hp.tile([128, DC, CR], BF16, tag="ost_r")
            for a, b in ch_r:
                for dc in range(DC):
                    ps_o = down_psum(wd_r, h_r, a, b, dc)
                    nc.vector.tensor_tensor(
                        ost_r[:, dc, a:b], ps_o[:], cwb[:, a:b], op=ALU.mult
                    )
            nc.sync.dma_start(outr_d[:], ost_r[:])
            for dc in range(DC):
                ps_o = down_psum(wd_s, h_s, 0, NS, dc)
                ost = op.tile([128, NS], BF16, tag="ost")
                if dc == DC - 1:
                    # last drain split across both engines so the final DMA
                    # issues sooner (a 2nd DMA here costs +12us — scheduler
                    # cliff — so both halves share one tile and one DMA)
                    nc.scalar.activation(
                        ost[:, 0 : NS // 2], ps_o[:, 0 : NS // 2], ACT.Copy
                    )
                    nc.vector.tensor_copy(ost[:, NS // 2 : NS], ps_o[:, NS // 2 : NS])
                elif dc % 2 == 0:
                    nc.scalar.activation(ost[:], ps_o[:], ACT.Copy)
                else:
                    nc.vector.tensor_copy(ost[:], ps_o[:])
                nc.sync.dma_start(outs_d[dc], ost[:])

    nc.compile()
    return nc


_NC_CACHE = {}


def _get_program(CR):
    if CR not in _NC_CACHE:
        _NC_CACHE[CR] = build_program(CR)
    return _NC_CACHE[CR]


def _route(xf, W_g):
    """Host top-2 routing: token lists + combine weights per expert."""
    logits = xf @ W_g                                   # [N, E] fp32
    l = logits.astype(np.float64)
    l -= l.max(axis=-1, keepdims=True)
    p = np.exp(l)
    p /= p.sum(axis=-1, keepdims=True)                  # fp64 softmax
    top2 = np.argsort(-logits, axis=-1, kind="stable")[:, :2]
    vals = np.take_along_axis(p, top2, axis=-1).astype(np.float32)
    idx, cw = [], []
    for e in range(E):
        mask = top2 == e                                # [N, 2]
        tok = np.nonzero(mask.any(axis=1))[0]
        w = vals[tok][mask[tok]]
        idx.append(tok)
        cw.append(w.astype(np.float32))
    return idx, cw


def _perm_x(m):
    """[1024, T] fp32 -> [128, DC, T] bf16 with row (c*128+p) at [p, c]."""
    return np.ascontiguousarray(
        m.astype(NPBF).reshape(DC, 128, -1).transpose(1, 0, 2)
    )


def _perm_w(m):
    """[1024, 512] -> [HC, 128, DC, 128]: [hc, p, dc, col] = m[dc*128+p, hc*128+col]."""
    return np.ascontiguousarray(
        m.astype(NPBF).reshape(DC, 128, HC, 128).transpose(2, 1, 0, 3)
    )


def _perm_wd(m):
    """[512, 1024] -> [128, HC, 1024]: [p, hc, d] = m[hc*128+p, d]."""
    return np.ascontiguousarray(
        m.astype(NPBF).reshape(HC, 128, D).transpose(1, 0, 2)
    )


def kernel(x, W_g, Wg_e, Wu_e, Wd_e, Wg_s, Wu_s, Wd_s, _trace=False, _trace_kwargs=None):
    x = np.asarray(x, dtype=np.float32)
    W_g = np.asarray(W_g, dtype=np.float32)
    Wg_e = np.asarray(Wg_e, dtype=np.float32)
    Wu_e = np.asarray(Wu_e, dtype=np.float32)
    Wd_e = np.asarray(Wd_e, dtype=np.float32)
    Wg_s = np.asarray(Wg_s, dtype=np.float32)
    Wu_s = np.asarray(Wu_s, dtype=np.float32)
    Wd_s = np.asarray(Wd_s, dtype=np.float32)

    B, T, _ = x.shape
    N = B * T
    xf = x.reshape(N, D)
    xT = np.ascontiguousarray(xf.T)                     # [D, N]

    idx, cw = _route(xf, W_g)
    CR = max(8, -(-max(len(i) for i in idx) // 8) * 8)
    nc = _get_program(CR)

    in_maps = []
    for c in range(N_CORES):
        e, half, q = c, c % 2, c // 2
        n_e = len(idx[e])
        xg = np.zeros((D, CR), dtype=np.float32)
        xg[:, :n_e] = xT[:, idx[e]]
        cwp = np.zeros((1, CR), dtype=np.float32)
        cwp[0, :n_e] = cw[e]
        in_maps.append(
            {
                "xg": _perm_x(xg),
                "xs": _perm_x(xT[:, q * NS : (q + 1) * NS]),
                "cw": cwp,
                "wg": np.stack(
                    [_perm_w(Wg_e[e]), _perm_w(Wg_s[:, half * DE : (half + 1) * DE])]
                ),
                "wu": np.stack(
                    [_perm_w(Wu_e[e]), _perm_w(Wu_s[:, half * DE : (half + 1) * DE])]
                ),
                "wd": np.stack(
                    [_perm_wd(Wd_e[e]), _perm_wd(Wd_s[half * DE : (half + 1) * DE, :])]
                ),
            }
        )

    res = run_bass_kernel_spmd(
        nc, in_maps, list(range(N_CORES)), trace=_trace, **(_trace_kwargs or {})
    )

    # ---- unshard: scatter-add partials into the full output ----
    out = np.zeros((N, D), dtype=np.float32)
    for c in range(N_CORES):
        e, q = c, c // 2
        n_e = len(idx[e])
        o_s = np.asarray(res.results[c]["outs"], dtype=np.float32).reshape(D, NS)
        out[q * NS : (q + 1) * NS, :] += o_s.T
        o_r = (
            np.asarray(res.results[c]["outr"], dtype=np.float32)
            .transpose(1, 0, 2)
            .reshape(D, CR)
        )
        out[idx[e], :] += o_r[:, :n_e].T
    result = out.reshape(B, T, D)
    if _trace:
        return result, res
    return result
